# revision 4
# baseline (speedup 1.0000x reference)
"""Trainium2 Bass kernel for 4-D valid convolution.

Problem: inputs [2, 64, 18, 18, 18, 18] fp32, kernel [81, 64, 64] fp32
(81 = 3^4 offsets row-major over (dw, dx, dy, dz)), output
[2, 64, 16, 16, 16, 16] fp32.

Sharding (8 cores): batch (2) x output-W chunks (4 chunks of 4).  Each core
receives an input slab x[b, :, w0:w0+6] plus the full kernel, and produces
out[b, :, w0:w0+4] as [64, 4, 16, 16, 16].

Per-core compute, per output tile-pair of 1024 positions (4 x-values x
16y x 16z, split qa/qb across PE column groups 0/64):
  - input is stored twice in SBUF: partitions 0-63 hold x at column c+1,
    partitions 64-127 hold x at column c.  Reading one column q therefore
    yields x[q-1] on the low half and x[q] on the high half - a built-in
    (+1 z) shift that lets one K=128 matmul cover offset pairs (dz=0, dz=1).
  - 27 such K=128 pair-matmuls + 27 K=64 matmuls for dz=2 (split across
    PE row-groups 0/64, 4-way quadrant packing) per tile-pair.
  - dtype float16: 1 col/cycle through the PE, 2 col-groups concurrent.

Layout/scheduling (this revision):
  - per-(slab, x0-window) SBUF tiles (24 of [128, ~1984]) loaded in compute
    order so the first matmul waits on only ~1.5 MB of DMA, not 4.5 MB.
  - ~10 warmup matmuls on the weight tile run during the input DMA so HAM
    un-throttles (1.2 -> 2.4 GHz) before real work starts.
  - merged PSUM banks: one [128,512] bank per tile-pair for pairs+even-dz2
    taps (pq), one for odd-dz2 taps (pr); epilogue = 1 ACT copy + 1 DVE add.
  - output DMAs round-robined over the sync/scalar/gpsimd queues; the last
    tile-pair's epilogue is split in column halves to shorten the tail.
"""

import numpy as np

B, CIN, COUT = 2, 64, 64
S = 18          # input spatial per dim
SO = 16         # output spatial per dim
NW = 4          # output w per core
NSLAB = 6       # input w slabs per core
SLAB = S * S * S          # 5832
XD = SLAB + 1             # duplicated layout incl. the +1-shift column
WIN = 1980                # x0-window width in columns ((x0+4)*324+684 - x0*324)
WCOLS = WIN + 12          # alloc width: room for the [q, q+648) view bound

# A/B flag: share LDWEIGHTS across the matmuls of a j-iteration via
# standalone ldweights + non-self-loading matmuls.
SHARED_LDW = False

_CACHE = {}


def _build_nc(dt_in):
    import concourse.bass as bass
    import concourse.mybir as mybir

    f32 = mybir.dt.float32

    nc = bass.Bass()
    # x pre-duplicated on host: rows 0-63 = slab shifted right by one column
    # (x[c-1]), rows 64-127 = slab (x[c]).
    x_h = nc.dram_tensor("x", [128, NSLAB, XD], dt_in, kind="ExternalInput")
    # weights pre-arranged on host to match SBUF layout exactly:
    # wkp[ci, j, co] = kernel[(dw,dx,dy)_j, dz=0..1]; wk1 = dz=2 duplicated.
    wkp_h = nc.dram_tensor("wkp", [128, 27, COUT], dt_in, kind="ExternalInput")
    wk1_h = nc.dram_tensor("wk1", [128, 27, COUT], dt_in, kind="ExternalInput")
    out_h = nc.dram_tensor(
        "out", [COUT, NW, SO, SO, SO], f32, kind="ExternalOutput"
    )

    tc = _make_tile_context(nc)
    with tc:
        with (
            tc.tile_pool(name="xp", bufs=1) as xpool,
            tc.tile_pool(name="wp", bufs=1) as wpool,
            tc.tile_pool(name="ob", bufs=3) as opool,
            tc.tile_pool(name="ps", bufs=2, space="PSUM") as ppool,
        ):
            dma_engines = [nc.sync, nc.scalar, nc.gpsimd]
            in_rr = [0]
            out_rr = [1]

            def load(dst, src):
                dma_engines[in_rr[0] % 3].dma_start(dst, src)
                in_rr[0] += 1

            def store(dst, src):
                dma_engines[out_rr[0] % 3].dma_start(dst, src)
                out_rr[0] += 1

            wp = wpool.tile([128, 27, COUT], dt_in, tag="wp")
            w1 = wpool.tile([128, 27, COUT], dt_in, tag="w1")
            load(wp[:], wkp_h[:])
            load(w1[:], wk1_h[:])

            # window tiles, allocated up front, loaded in compute order
            wt = [[None] * NW for _ in range(NSLAB)]
            for s in range(NSLAB):
                for xi in range(NW):
                    wt[s][xi] = xpool.tile(
                        [128, WCOLS], dt_in, name=f"x{s}_{xi}",
                        tag=f"x{s}_{xi}",
                    )

            def load_win(s, xi):
                c0 = xi * 4 * 324
                wdt = min(WIN, XD - c0)
                load(wt[s][xi][:, 0:wdt], x_h[:, s, c0 : c0 + wdt])

            # order matches the tile-pair groups below
            for s, xi in (
                (0, 0), (0, 1), (1, 0), (1, 1), (2, 0), (2, 1),
                (0, 2), (0, 3), (1, 2), (1, 3), (2, 2), (2, 3),
                (3, 0), (3, 1), (3, 2), (3, 3),
                (4, 0), (4, 1), (4, 2), (4, 3),
                (5, 0), (5, 1), (5, 2), (5, 3),
            ):
                load_win(s, xi)

            def rhs(xt, prange, q0):
                # [p, 2x, 16y, 16z] view with steps (324, 18, 1) at column q0
                v = xt[prange, q0 : q0 + 648]
                v = v.rearrange("p (x y z) -> p x y z", x=2, y=18, z=18)
                return v[:, :, 0:16, 0:16]

            PFULL = slice(0, 128)
            PLO = slice(0, 64)
            PHI = slice(64, 128)

            def set_noload(m):
                try:
                    (m.ins if hasattr(m, "ins") else m).ldweights = False
                except Exception:
                    pass

            # ---- PE warmup: dummy matmuls on the weight tile while the
            # input DMA streams in, so HAM reaches 8/8 before real work.
            warm_ps = ppool.tile([128, 512], f32, tag="pqa")
            wrhs = wp[:, 0:8, :].rearrange("p a b -> p (a b)")
            for _ in range(10):
                nc.tensor.matmul(
                    warm_ps[0:64, :], wp[:, 0, :], wrhs,
                    start=True, stop=True, tile_position=(0, 0),
                )

            # ---- main loop: 8 groups of 2 tile-pairs ----
            groups = []
            for w in range(NW):
                groups.append([(w, 0), (w, 1)])
                groups.append([(w, 2), (w, 3)])

            n_tp = 0
            for g, tps in enumerate(groups):
                banks = []
                for t, (w, xi) in enumerate(tps):
                    pq = ppool.tile([128, 512], f32, tag=f"pq{'ab'[t]}")
                    pr = ppool.tile([128, 512], f32, tag=f"pr{'ab'[t]}")
                    banks.append((pq, pr))

                # pairs phase: 27 K=128 matmuls per output tile
                for j in range(27):
                    dw, dx, dy = j // 9, (j // 3) % 3, j % 3
                    qa = 1 + dx * 324 + dy * 18
                    qb = qa + 648
                    if SHARED_LDW:
                        nc.tensor.ldweights(
                            wp[:, j, :], tile_position=(0, 0)
                        )
                        nc.tensor.ldweights(
                            wp[:, j, :], tile_position=(0, 64)
                        )
                    st = j == 0
                    for t, (w, xi) in enumerate(tps):
                        xt = wt[w + dw][xi]
                        pq = banks[t][0]
                        m1 = nc.tensor.matmul(
                            pq[0:64, :], wp[:, j, :], rhs(xt, PFULL, qa),
                            start=st, stop=False, tile_position=(0, 0),
                        )
                        m2 = nc.tensor.matmul(
                            pq[64:128, :], wp[:, j, :], rhs(xt, PFULL, qb),
                            start=st, stop=False, tile_position=(0, 64),
                        )
                        if SHARED_LDW:
                            set_noload(m1)
                            set_noload(m2)

                # singles phase: dz=2, K=64, 4-way quadrant packing.
                # even j -> rows 0-63 -> pq (continues pairs accumulation);
                # odd j -> rows 64-127 -> pr.
                n_idx = 14
                for idx in range(n_idx):
                    je = 2 * idx
                    jo = je + 1
                    if SHARED_LDW:
                        nc.tensor.ldweights(
                            w1[0:64, je, :], tile_position=(0, 0)
                        )
                        nc.tensor.ldweights(
                            w1[0:64, je, :], tile_position=(0, 64)
                        )
                        if jo < 27:
                            nc.tensor.ldweights(
                                w1[64:128, jo, :], tile_position=(64, 0)
                            )
                            nc.tensor.ldweights(
                                w1[64:128, jo, :], tile_position=(64, 64)
                            )
                    for t, (w, xi) in enumerate(tps):
                        pq, pr = banks[t]
                        # parity 0 (even j): rows 0-63 read x[q-1] -> q=base+1
                        dw, dx, dy = je // 9, (je // 3) % 3, je % 3
                        xt = wt[w + dw][xi]
                        q0 = dx * 324 + dy * 18 + 2 + 1
                        stop0 = idx == n_idx - 1
                        m1 = nc.tensor.matmul(
                            pq[0:64, :], w1[0:64, je, :], rhs(xt, PLO, q0),
                            start=False, stop=stop0, tile_position=(0, 0),
                        )
                        m2 = nc.tensor.matmul(
                            pq[64:128, :], w1[0:64, je, :],
                            rhs(xt, PLO, q0 + 648),
                            start=False, stop=stop0, tile_position=(0, 64),
                        )
                        ms = [m1, m2]
                        if jo < 27:
                            # parity 1 (odd j): rows 64-127 read x[q]
                            dw, dx, dy = jo // 9, (jo // 3) % 3, jo % 3
                            xt = wt[w + dw][xi]
                            q1 = dx * 324 + dy * 18 + 2
                            st1 = idx == 0
                            stop1 = idx == n_idx - 2
                            m3 = nc.tensor.matmul(
                                pr[0:64, :], w1[64:128, jo, :],
                                rhs(xt, PHI, q1),
                                start=st1, stop=stop1,
                                tile_position=(64, 0),
                            )
                            m4 = nc.tensor.matmul(
                                pr[64:128, :], w1[64:128, jo, :],
                                rhs(xt, PHI, q1 + 648),
                                start=st1, stop=stop1,
                                tile_position=(64, 64),
                            )
                            ms += [m3, m4]
                        if SHARED_LDW:
                            for m in ms:
                                set_noload(m)

                # epilogue: osb = pr (ACT copy, single PSUM read) + pq
                # (DVE add, single PSUM read), then DMA out.
                for t, (w, xi) in enumerate(tps):
                    pq, pr = banks[t]
                    n_tp += 1
                    last = n_tp == 16
                    x0 = 4 * xi
                    osb = opool.tile([128, 512], f32, tag="osb")
                    if not last:
                        nc.scalar.copy(osb[:, :], pr[:, :])
                        nc.vector.tensor_add(
                            out=osb[:, :], in0=pq[:, :], in1=osb[:, :]
                        )
                        lo = osb[0:64, :].rearrange(
                            "p (x y z) -> p x y z", x=2, y=16, z=16
                        )
                        hi = osb[64:128, :].rearrange(
                            "p (x y z) -> p x y z", x=2, y=16, z=16
                        )
                        store(out_h[:, w, x0 : x0 + 2, :, :], lo)
                        store(out_h[:, w, x0 + 2 : x0 + 4, :, :], hi)
                    else:
                        # split the last epilogue in column halves so the
                        # kernel tail is one half-epilogue, not a full one
                        for h in range(2):
                            cs = slice(256 * h, 256 * h + 256)
                            nc.scalar.copy(osb[:, cs], pr[:, cs])
                            nc.vector.tensor_add(
                                out=osb[:, cs], in0=pq[:, cs], in1=osb[:, cs]
                            )
                            lo = osb[0:64, cs].rearrange(
                                "p (y z) -> p y z", y=16, z=16
                            )
                            hi = osb[64:128, cs].rearrange(
                                "p (y z) -> p y z", y=16, z=16
                            )
                            store(out_h[:, w, x0 + h, :, :], lo)
                            store(out_h[:, w, x0 + 2 + h, :, :], hi)

    _dedup_ldweights(nc)
    _split_multiwaits(nc)
    return nc


def _dedup_ldweights(nc):
    """bass lowers every matmul to a standalone InstLdweights + a
    non-self-loading InstMatmult.  Consecutive matmuls in a group share the
    same stationary weights per PE quadrant, so half those loads reload
    identical data.  Delete an InstLdweights when the same (AP, quadrant)
    load is still resident and nothing overlapping was loaded in between.
    Loads carrying sync waits/updates are kept (sem graph untouched)."""
    removed = 0
    for fn in nc.m.functions:
        for blk in fn.blocks:
            resident = {}  # (r0, c0, r1, c1) -> signature
            out = []
            for inst in blk.instructions:
                if type(inst).__name__ != "InstLdweights":
                    out.append(inst)
                    continue
                pos = inst.tile_position or (0, 0)
                ts = inst.tile_size or (128, 128)
                reg = (pos[0], pos[1], pos[0] + ts[0], pos[1] + ts[1])
                sig = (repr(inst.ins[0]), pos, ts, inst.perf_mode)
                si = inst.sync_info
                clean = si is None or (not si.on_wait and not si.on_update)
                if clean and resident.get(reg) == sig:
                    removed += 1
                    continue
                # invalidate anything overlapping this load's region
                for r in list(resident):
                    if (
                        r[0] < reg[2] and reg[0] < r[2]
                        and r[1] < reg[3] and reg[1] < r[3]
                    ):
                        del resident[r]
                resident[reg] = sig
                out.append(inst)
            blk.instructions = out
    return removed


def _make_tile_context(nc):
    from concourse.tile import TileContext

    class TC(TileContext):
        # stock teardown is drain -> barrier -> sem-clear -> barrier; the
        # final barrier only orders engine-stream ends and costs ~2us.
        def _drain_and_barrier(self, tick_clock, wait_clock):
            from concourse.vector_clock import ScopedClock

            nc = self.nc
            drain_inst = nc.sync.drain()
            wait_clock.add_sem_waits(
                drain_inst.ins, ScopedClock({None: tick_clock.global_clock})
            )
            nc.all_engine_barrier()
            assert self.sems is not None
            popped = nc._tile_sem_poison_stack.pop()
            assert popped is self._sem_poison
            nc.clear_and_free_semaphores(list(self.sems.allocated().values()))

    return TC(nc)


def _split_multiwaits(nc, max_waits=1):
    """The walrus build here rejects any instruction carrying more than one
    sync-wait ("Too many sync wait commands").  Tile attaches one wait per
    outstanding producer.  Move excess waits onto same-engine NoOps inserted
    immediately before the instruction - semantically identical."""
    import concourse.mybir as mybir

    n_split = 0
    for fn in nc.m.functions:
        for blk in fn.blocks:
            out = []
            for inst in list(blk.instructions):
                si = inst.sync_info
                if si is not None and si.on_wait and len(si.on_wait) > max_waits:
                    waits = list(si.on_wait)
                    extra = waits[:-max_waits]
                    for k in range(0, len(extra), max_waits):
                        nop = mybir.InstNoOp(
                            name=f"{inst.name}.w{k}", ins=[], outs=[]
                        )
                        nop.engine = inst.engine
                        nop.sync_info = mybir.SyncInfo(
                            on_wait=extra[k : k + max_waits], on_update=[]
                        )
                        nc.register_instruction(nop)
                        out.append(nop)
                        n_split += 1
                    si.on_wait = waits[-max_waits:]
                out.append(inst)
            blk.instructions = out
    return n_split


# compute dtype: "float16" (fastest, rel err ~3e-4) or "float32r"
DTYPE = "float16"


def _get_nc():
    if "nc" not in _CACHE:
        import concourse.mybir as mybir

        _CACHE["nc"] = _build_nc(getattr(mybir.dt, DTYPE))
    return _CACHE["nc"]


def _np_dtype():
    if DTYPE == "float16":
        return np.float16
    return np.float32


def _shard_inputs(inputs):
    nd = _np_dtype()
    x = np.asarray(inputs["inputs"], dtype=np.float32).astype(nd)
    wk = np.asarray(inputs["kernel"], dtype=np.float32).astype(nd)
    k3 = wk.reshape(27, 3, CIN, COUT)  # [j, dz, ci, co]
    wkp = np.ascontiguousarray(
        np.concatenate(
            [k3[:, 0].transpose(1, 0, 2), k3[:, 1].transpose(1, 0, 2)], axis=0
        )
    )
    w1h = k3[:, 2].transpose(1, 0, 2)
    wk1 = np.ascontiguousarray(np.concatenate([w1h, w1h], axis=0))
    in_maps = []
    for c in range(8):
        b, wc = c // 4, c % 4
        w0 = 4 * wc
        slab = x[b, :, w0 : w0 + 6].reshape(CIN, NSLAB, SLAB)
        dup = np.zeros((128, NSLAB, XD), dtype=nd)
        dup[0:CIN, :, 1:XD] = slab            # lo rows: x[c-1]
        dup[CIN:, :, 0:SLAB] = slab           # hi rows: x[c]
        in_maps.append({"x": dup, "wkp": wkp, "wk1": wk1})
    return in_maps


def _gather_outputs(results):
    out = np.empty((B, COUT, NW * 4, SO, SO, SO), dtype=np.float32)
    for c in range(8):
        b, wc = c // 4, c % 4
        w0 = 4 * wc
        out[b, :, w0 : w0 + 4] = results[c]["out"]
    return out


def kernel(**inputs):
    from concourse.bass_utils import run_bass_kernel_spmd

    res = run_bass_kernel_spmd(_get_nc(), _shard_inputs(inputs), list(range(8)))
    return _gather_outputs(res.results)


# revision 5
# speedup vs baseline: 1.1242x; 1.1242x over previous
"""Trainium2 Bass kernel for 4-D valid convolution.

Problem: inputs [2, 64, 18, 18, 18, 18] fp32, kernel [81, 64, 64] fp32
(81 = 3^4 offsets row-major over (dw, dx, dy, dz)), output
[2, 64, 16, 16, 16, 16] fp32.

Sharding (8 cores): batch (2) x output-W chunks (4 chunks of 4).  Each core
receives an input slab x[b, :, w0:w0+6] plus the full kernel, and produces
out[b, :, w0:w0+4] as [64, 4, 16, 16, 16].

All matmul reads are CONTIGUOUS via two host-compacted layouts per slab
(measured on HW: a strided conv window costs +27 ns per 512-col slot; a
contiguous one runs at the 215.8 ns floor):

  Layout A [128, 18x * (18y*16z)]: rows 64-127 hold x(..., z+0) z-compacted
  to 16, rows 0-63 hold x(..., z+1).  One K=128 matmul at column base
  ((x0+dx)*288 + dy*16) covers the offset pair (dz=0, dz=1) for tap
  (dw,dx,dy) with a fully contiguous [p, 2x, 256] read.  27 taps.

  Layout B [128, 18x * (19y'*16z)]: z-base 2 compacted (cb = x(...,z+2)),
  rows 64-127 = cb(y'), rows 0-63 = cb(y'-1) (one-row y shift, 19-row
  panels so the shift never clips).  One K=128 matmul at y-base 1 covers
  the pair (dy=0,dz=2)+(dy=1,dz=2) per (dw,dx): 9 taps-pairs.  The 9
  leftover (dy=2,dz=2) taps run as K=64 quad-packed singles (even tap on
  rows 0-63 at y-base 3, odd on rows 64-127 at y-base 2).

Per output tile-pair (1024 positions = 4 x-values x 16y x 16z, split
qa/qb across PE column groups 0/64): 27 + 9 K=128 slots + ~5 quad slots,
one PSUM bank for pairs+even-singles (pq), one for odd singles (pr).
Groups of 2 tile-pairs share LDWEIGHTS (a post-pass deletes reloads of
identical resident weights; measured: <=1 LDW per 512-col slot is free).

Scheduling: slab tiles with column-chunked DMA (dep tracking is
range-based), loads ordered by first use; ~12 warmup matmuls on a memset
scratch tile un-throttle the PE clock (1.2 -> 2.4 GHz) during the DMA
prologue; output DMAs ride the otherwise-idle gpsimd queue except the
last tile-pair's, which use the low-latency HWDGE queues.
"""

import numpy as np

B, CIN, COUT = 2, 64, 64
S = 18          # input spatial per dim
SO = 16         # output spatial per dim
NW = 4          # output w per core
NSLAB = 6       # input w slabs per core
PA = 288        # layout A x-panel: 18y * 16z
PB = 304        # layout B x-panel: 19y' * 16z
ACOLS = 18 * PA          # 5184
BCOLS = 18 * PB          # 5472
AALLOC = ACOLS + 40      # view-extent padding (max extent 5216)
BALLOC = BCOLS + 52      # max extent 5520

_CACHE = {}


def _build_nc(dt_in):
    import concourse.bass as bass
    import concourse.mybir as mybir

    f32 = mybir.dt.float32

    nc = bass.Bass()
    xa_h = nc.dram_tensor("xa", [128, NSLAB, ACOLS], dt_in, kind="ExternalInput")
    xb_h = nc.dram_tensor("xb", [128, NSLAB, BCOLS], dt_in, kind="ExternalInput")
    wa_h = nc.dram_tensor("wa", [128, 27, COUT], dt_in, kind="ExternalInput")
    wb_h = nc.dram_tensor("wb", [128, 9, COUT], dt_in, kind="ExternalInput")
    w1_h = nc.dram_tensor("w1", [128, 5, COUT], dt_in, kind="ExternalInput")
    out_h = nc.dram_tensor(
        "out", [COUT, NW, SO, SO, SO], f32, kind="ExternalOutput"
    )

    tc = _make_tile_context(nc)
    with tc:
        with (
            tc.tile_pool(name="xp", bufs=1) as xpool,
            tc.tile_pool(name="wp", bufs=1) as wpool,
            tc.tile_pool(name="ob", bufs=3) as opool,
            tc.tile_pool(name="ps", bufs=2, space="PSUM") as ppool,
        ):
            # inputs on the two HWDGE queues; outputs mostly on gpsimd
            in_e = [nc.sync, nc.scalar]
            in_rr = [0]

            def load(dst, src):
                in_e[in_rr[0] % 2].dma_start(dst, src)
                in_rr[0] += 1

            wa = wpool.tile([128, 27, COUT], dt_in, tag="wa")
            wb = wpool.tile([128, 9, COUT], dt_in, tag="wb")
            w1 = wpool.tile([128, 5, COUT], dt_in, tag="w1")

            xat = [
                xpool.tile([128, AALLOC], dt_in, name=f"xa{s}", tag=f"xa{s}")
                for s in range(NSLAB)
            ]
            xbt = [
                xpool.tile([128, BALLOC], dt_in, name=f"xb{s}", tag=f"xb{s}")
                for s in range(NSLAB)
            ]

            # thirds, aligned so tile-pair xi needs chunks {xi<=1: T0/T1,
            # xi>=2: T1/T2} of slabs w..w+2
            AT = [(0, 6 * PA), (6 * PA, 12 * PA), (12 * PA, 18 * PA)]
            BT = [(0, 6 * PB), (6 * PB, 12 * PB), (12 * PB, 18 * PB)]

            def load_a(s, t):
                c0, c1 = AT[t]
                load(xat[s][:, c0:c1], xa_h[:, s, c0:c1])

            def load_b(s, t):
                c0, c1 = BT[t]
                load(xbt[s][:, c0:c1], xb_h[:, s, c0:c1])

            load(wa[:], wa_h[:])
            load(wb[:], wb_h[:])
            load(w1[:], w1_h[:])
            for s in (0, 1, 2):
                load_a(s, 0)
            for s in (0, 1, 2):
                load_a(s, 1)
            for s in (0, 1, 2):
                load_b(s, 0)
                load_b(s, 1)
            for s in (0, 1, 2):
                load_a(s, 2)
                load_b(s, 2)
            for s in (3, 4, 5):
                for t in (0, 1, 2):
                    load_a(s, t)
                    load_b(s, t)

            def rhs_a(xt, prange, q0):
                v = xt[prange, q0 : q0 + 2 * PA]
                v = v.rearrange("p (x c) -> p x c", x=2, c=PA)
                return v[:, :, 0:256]

            def rhs_b(xt, prange, q0):
                v = xt[prange, q0 : q0 + 2 * PB]
                v = v.rearrange("p (x c) -> p x c", x=2, c=PB)
                return v[:, :, 0:256]

            PFULL = slice(0, 128)
            PLO = slice(0, 64)
            PHI = slice(64, 128)

            # ---- PE warmup on a memset scratch tile (no DMA dependency)
            scr = xpool.tile([128, 576], dt_in, name="scr", tag="scr")
            nc.gpsimd.memset(scr[:], 0.0)
            warm_ps = ppool.tile([128, 512], f32, tag="pqa")
            for _ in range(12):
                nc.tensor.matmul(
                    warm_ps[0:64, :], scr[:, 0:64], scr[:, 64:576],
                    start=True, stop=True, tile_position=(0, 0),
                )

            # ---- main loop: 8 groups of 2 tile-pairs ----
            groups = []
            for w in range(NW):
                groups.append([(w, 0), (w, 1)])
                groups.append([(w, 2), (w, 3)])

            n_tp = 0
            for g, tps in enumerate(groups):
                banks = []
                for t in range(len(tps)):
                    pq = ppool.tile([128, 512], f32, tag=f"pq{'ab'[t]}")
                    pr = ppool.tile([128, 512], f32, tag=f"pr{'ab'[t]}")
                    banks.append((pq, pr))

                # A-pairs: 27 K=128 matmuls per tile, offsets (dw,dx,dy,dz 0|1)
                for j in range(27):
                    dw, dx, dy = j // 9, (j // 3) % 3, j % 3
                    st = j == 0
                    for t, (w, xi) in enumerate(tps):
                        xt = xat[w + dw]
                        qa = (4 * xi + dx) * PA + dy * 16
                        pq = banks[t][0]
                        nc.tensor.matmul(
                            pq[0:64, :], wa[:, j, :], rhs_a(xt, PFULL, qa),
                            start=st, stop=False, tile_position=(0, 0),
                        )
                        nc.tensor.matmul(
                            pq[64:128, :], wa[:, j, :],
                            rhs_a(xt, PFULL, qa + 2 * PA),
                            start=st, stop=False, tile_position=(0, 64),
                        )

                # B-pairs: 9 K=128 matmuls per tile, (dw,dx, dy 0|1, dz=2)
                for m in range(9):
                    dw, dx = m // 3, m % 3
                    for t, (w, xi) in enumerate(tps):
                        xt = xbt[w + dw]
                        qa = (4 * xi + dx) * PB + 16
                        pq = banks[t][0]
                        nc.tensor.matmul(
                            pq[0:64, :], wb[:, m, :], rhs_b(xt, PFULL, qa),
                            start=False, stop=False, tile_position=(0, 0),
                        )
                        nc.tensor.matmul(
                            pq[64:128, :], wb[:, m, :],
                            rhs_b(xt, PFULL, qa + 2 * PB),
                            start=False, stop=False, tile_position=(0, 64),
                        )

                # B-singles: (dw,dx, dy=2, dz=2), K=64 quad-packed
                for i in range(5):
                    me = 2 * i
                    mo = me + 1
                    for t, (w, xi) in enumerate(tps):
                        pq, pr = banks[t]
                        dw, dx = me // 3, me % 3
                        xt = xbt[w + dw]
                        qe = (4 * xi + dx) * PB + 48
                        stop0 = i == 4
                        nc.tensor.matmul(
                            pq[0:64, :], w1[0:64, i, :], rhs_b(xt, PLO, qe),
                            start=False, stop=stop0, tile_position=(0, 0),
                        )
                        nc.tensor.matmul(
                            pq[64:128, :], w1[0:64, i, :],
                            rhs_b(xt, PLO, qe + 2 * PB),
                            start=False, stop=stop0, tile_position=(0, 64),
                        )
                        if mo < 9:
                            dw, dx = mo // 3, mo % 3
                            xt = xbt[w + dw]
                            qo = (4 * xi + dx) * PB + 32
                            st1 = i == 0
                            stop1 = i == 3
                            nc.tensor.matmul(
                                pr[0:64, :], w1[64:128, i, :],
                                rhs_b(xt, PHI, qo),
                                start=st1, stop=stop1,
                                tile_position=(64, 0),
                            )
                            nc.tensor.matmul(
                                pr[64:128, :], w1[64:128, i, :],
                                rhs_b(xt, PHI, qo + 2 * PB),
                                start=st1, stop=stop1,
                                tile_position=(64, 64),
                            )

                # epilogue: osb = pr (ACT copy) + pq (DVE add), then DMA out
                for t, (w, xi) in enumerate(tps):
                    pq, pr = banks[t]
                    n_tp += 1
                    x0 = 4 * xi
                    osb = opool.tile([128, 512], f32, tag="osb")
                    nc.scalar.copy(osb[:, :], pr[:, :])
                    nc.vector.tensor_add(
                        out=osb[:, :], in0=pq[:, :], in1=osb[:, :]
                    )
                    lo = osb[0:64, :].rearrange(
                        "p (x y z) -> p x y z", x=2, y=16, z=16
                    )
                    hi = osb[64:128, :].rearrange(
                        "p (x y z) -> p x y z", x=2, y=16, z=16
                    )
                    if n_tp == 16:
                        nc.sync.dma_start(out_h[:, w, x0 : x0 + 2, :, :], lo)
                        nc.scalar.dma_start(
                            out_h[:, w, x0 + 2 : x0 + 4, :, :], hi
                        )
                    else:
                        nc.gpsimd.dma_start(out_h[:, w, x0 : x0 + 2, :, :], lo)
                        nc.gpsimd.dma_start(
                            out_h[:, w, x0 + 2 : x0 + 4, :, :], hi
                        )

    _dedup_ldweights(nc)
    _split_multiwaits(nc)
    return nc


def _dedup_ldweights(nc):
    """bass lowers every matmul to a standalone InstLdweights + a
    non-self-loading InstMatmult.  Consecutive matmuls in a group share the
    same stationary weights per PE quadrant, so half those loads reload
    identical data.  Delete an InstLdweights when the same (AP, quadrant)
    load is still resident and nothing overlapping was loaded in between.
    Loads carrying sync waits/updates are kept (sem graph untouched)."""
    removed = 0
    for fn in nc.m.functions:
        for blk in fn.blocks:
            resident = {}  # (r0, c0, r1, c1) -> signature
            out = []
            for inst in blk.instructions:
                if type(inst).__name__ != "InstLdweights":
                    out.append(inst)
                    continue
                pos = inst.tile_position or (0, 0)
                ts = inst.tile_size or (128, 128)
                reg = (pos[0], pos[1], pos[0] + ts[0], pos[1] + ts[1])
                sig = (repr(inst.ins[0]), pos, ts, inst.perf_mode)
                si = inst.sync_info
                clean = si is None or (not si.on_wait and not si.on_update)
                if clean and resident.get(reg) == sig:
                    removed += 1
                    continue
                # invalidate anything overlapping this load's region
                for r in list(resident):
                    if (
                        r[0] < reg[2] and reg[0] < r[2]
                        and r[1] < reg[3] and reg[1] < r[3]
                    ):
                        del resident[r]
                resident[reg] = sig
                out.append(inst)
            blk.instructions = out
    return removed


def _make_tile_context(nc):
    from concourse.tile import TileContext

    class TC(TileContext):
        # stock teardown is drain -> barrier -> sem-clear -> barrier; the
        # final barrier only orders engine-stream ends and costs ~2us.
        def _drain_and_barrier(self, tick_clock, wait_clock):
            from concourse.vector_clock import ScopedClock

            nc = self.nc
            drain_inst = nc.sync.drain()
            wait_clock.add_sem_waits(
                drain_inst.ins, ScopedClock({None: tick_clock.global_clock})
            )
            nc.all_engine_barrier()
            assert self.sems is not None
            popped = nc._tile_sem_poison_stack.pop()
            assert popped is self._sem_poison
            nc.clear_and_free_semaphores(list(self.sems.allocated().values()))

    return TC(nc)


def _split_multiwaits(nc, max_waits=1):
    """The walrus build here rejects any instruction carrying more than one
    sync-wait ("Too many sync wait commands").  Tile attaches one wait per
    outstanding producer.  Move excess waits onto same-engine NoOps inserted
    immediately before the instruction - semantically identical."""
    import concourse.mybir as mybir

    n_split = 0
    for fn in nc.m.functions:
        for blk in fn.blocks:
            out = []
            for inst in list(blk.instructions):
                si = inst.sync_info
                if si is not None and si.on_wait and len(si.on_wait) > max_waits:
                    waits = list(si.on_wait)
                    extra = waits[:-max_waits]
                    for k in range(0, len(extra), max_waits):
                        nop = mybir.InstNoOp(
                            name=f"{inst.name}.w{k}", ins=[], outs=[]
                        )
                        nop.engine = inst.engine
                        nop.sync_info = mybir.SyncInfo(
                            on_wait=extra[k : k + max_waits], on_update=[]
                        )
                        nc.register_instruction(nop)
                        out.append(nop)
                        n_split += 1
                    si.on_wait = waits[-max_waits:]
                out.append(inst)
            blk.instructions = out
    return n_split


# compute dtype: "float16" (fastest, rel err ~3e-4)
DTYPE = "float16"


def _get_nc():
    if "nc" not in _CACHE:
        import concourse.mybir as mybir

        _CACHE["nc"] = _build_nc(getattr(mybir.dt, DTYPE))
    return _CACHE["nc"]


def _np_dtype():
    if DTYPE == "float16":
        return np.float16
    return np.float32


def _shard_inputs(inputs):
    nd = _np_dtype()
    x = np.asarray(inputs["inputs"], dtype=np.float32).astype(nd)
    wk = np.asarray(inputs["kernel"], dtype=np.float32).astype(nd)
    k4 = wk.reshape(3, 3, 3, 3, CIN, COUT)  # [dw,dx,dy,dz,ci,co]

    # layout A weights: rows 0-63 = dz=1 tap, rows 64-127 = dz=0 tap
    wa = np.empty((128, 27, COUT), dtype=nd)
    wa[0:64] = k4[:, :, :, 1].reshape(27, CIN, COUT).transpose(1, 0, 2)
    wa[64:128] = k4[:, :, :, 0].reshape(27, CIN, COUT).transpose(1, 0, 2)
    # layout B pair weights: rows 0-63 = (dy=0,dz=2), rows 64-127 = (dy=1,dz=2)
    wb = np.empty((128, 9, COUT), dtype=nd)
    wb[0:64] = k4[:, :, 0, 2].reshape(9, CIN, COUT).transpose(1, 0, 2)
    wb[64:128] = k4[:, :, 1, 2].reshape(9, CIN, COUT).transpose(1, 0, 2)
    # B singles (dy=2,dz=2): even tap index on rows 0-63, odd on rows 64-127
    w22 = k4[:, :, 2, 2].reshape(9, CIN, COUT)
    w1 = np.zeros((128, 5, COUT), dtype=nd)
    w1[0:64] = w22[0::2].transpose(1, 0, 2)
    w1[64:128, 0:4] = w22[1::2].transpose(1, 0, 2)

    in_maps = []
    for c in range(8):
        b, wc = c // 4, c % 4
        w0 = 4 * wc
        slab = x[b, :, w0 : w0 + 6]  # [64, 6, 18, 18, 18] (w, x, y, z)
        # layout A: z compacted to 16; lo rows z-base 1, hi rows z-base 0
        xa = np.empty((128, NSLAB, ACOLS), dtype=nd)
        xa[0:64] = slab[..., 1:17].reshape(CIN, NSLAB, ACOLS)
        xa[64:128] = slab[..., 0:16].reshape(CIN, NSLAB, ACOLS)
        # layout B: z-base 2 compacted, 19-row y panels; lo rows shifted +1 y
        cb = slab[..., 2:18]  # [64, 6, 18, 18, 16]
        lo = np.zeros((CIN, NSLAB, S, 19, 16), dtype=nd)
        hi = np.zeros((CIN, NSLAB, S, 19, 16), dtype=nd)
        lo[:, :, :, 1:19] = cb
        hi[:, :, :, 0:18] = cb
        xb = np.empty((128, NSLAB, BCOLS), dtype=nd)
        xb[0:64] = lo.reshape(CIN, NSLAB, BCOLS)
        xb[64:128] = hi.reshape(CIN, NSLAB, BCOLS)
        in_maps.append({"xa": xa, "xb": xb, "wa": wa, "wb": wb, "w1": w1})
    return in_maps


def _gather_outputs(results):
    out = np.empty((B, COUT, NW * 4, SO, SO, SO), dtype=np.float32)
    for c in range(8):
        b, wc = c // 4, c % 4
        w0 = 4 * wc
        out[b, :, w0 : w0 + 4] = results[c]["out"]
    return out


def kernel(**inputs):
    from concourse.bass_utils import run_bass_kernel_spmd

    res = run_bass_kernel_spmd(_get_nc(), _shard_inputs(inputs), list(range(8)))
    return _gather_outputs(res.results)


# revision 10
# speedup vs baseline: 1.1395x; 1.0135x over previous
"""Trainium2 Bass kernel for 4-D valid convolution.

Problem: inputs [2, 64, 18, 18, 18, 18] fp32, kernel [81, 64, 64] fp32
(81 = 3^4 offsets row-major over (dw, dx, dy, dz)), output
[2, 64, 16, 16, 16, 16] fp32.

Sharding (8 cores): batch (2) x output-W chunks (4 chunks of 4).  Each core
receives an input slab x[b, :, w0:w0+6] plus the full kernel, and produces
out[b, :, w0:w0+4] as [64, 4, 16, 16, 16].

All matmul reads are CONTIGUOUS via two host-compacted layouts per slab
(measured on HW: a strided conv window costs +27 ns per 512-col slot; a
contiguous one runs at the 215.8 ns floor):

  Layout A [128, 18x * (18y*16z)]: rows 64-127 hold x(..., z+0) z-compacted
  to 16, rows 0-63 hold x(..., z+1).  One K=128 matmul at column base
  ((x0+dx)*288 + dy*16) covers the offset pair (dz=0, dz=1) for tap
  (dw,dx,dy) with a fully contiguous [p, 2x, 256] read.  27 taps.

  Layout B [128, 18x * (19y'*16z)]: z-base 2 compacted (cb = x(...,z+2)),
  rows 64-127 = cb(y'), rows 0-63 = cb(y'-1) (one-row y shift, 19-row
  panels so the shift never clips).  One K=128 matmul at y-base 1 covers
  the pair (dy=0,dz=2)+(dy=1,dz=2) per (dw,dx): 9 taps-pairs.  The 9
  leftover (dy=2,dz=2) taps run as K=64 quad-packed singles (even tap on
  rows 0-63 at y-base 3, odd on rows 64-127 at y-base 2).

Per output tile-pair (1024 positions = 4 x-values x 16y x 16z, split
qa/qb across PE column groups 0/64): 27 + 9 K=128 slots + ~5 quad slots,
one PSUM bank for pairs+even-singles (pq), one for odd singles (pr).
Groups of 2 tile-pairs share LDWEIGHTS (a post-pass deletes reloads of
identical resident weights; measured: <=1 LDW per 512-col slot is free).

Scheduling: slab tiles with column-chunked DMA (dep tracking is
range-based), loads ordered by first use; ~12 warmup matmuls on a memset
scratch tile un-throttle the PE clock (1.2 -> 2.4 GHz) during the DMA
prologue; output DMAs ride the otherwise-idle gpsimd queue except the
last tile-pair's, which use the low-latency HWDGE queues.
"""

import numpy as np

B, CIN, COUT = 2, 64, 64
S = 18          # input spatial per dim
SO = 16         # output spatial per dim
NW = 4          # output w per core
NSLAB = 6       # input w slabs per core
PA = 288        # layout A x-panel: 18y * 16z
PB = 304        # layout B x-panel: 19y' * 16z
ACOLS = 18 * PA          # 5184
BCOLS = 18 * PB          # 5472
AALLOC = ACOLS + 40      # view-extent padding (max extent 5216)
BALLOC = BCOLS + 52      # max extent 5520

_CACHE = {}


def _build_nc(dt_in):
    import concourse.bass as bass
    import concourse.mybir as mybir

    f32 = mybir.dt.float32

    nc = bass.Bass()
    xa_h = nc.dram_tensor("xa", [128, NSLAB, ACOLS], dt_in, kind="ExternalInput")
    xb_h = nc.dram_tensor("xb", [128, NSLAB, BCOLS], dt_in, kind="ExternalInput")
    wa_h = nc.dram_tensor("wa", [128, 27, COUT], dt_in, kind="ExternalInput")
    wb_h = nc.dram_tensor("wb", [128, 9, COUT], dt_in, kind="ExternalInput")
    w1_h = nc.dram_tensor("w1", [128, 5, COUT], dt_in, kind="ExternalInput")
    out_h = nc.dram_tensor(
        "out", [COUT, NW, SO, SO, SO], dt_in, kind="ExternalOutput"
    )

    tc = _make_tile_context(nc)
    with tc:
        with (
            tc.tile_pool(name="xp", bufs=1) as xpool,
            tc.tile_pool(name="wp", bufs=1) as wpool,
            tc.tile_pool(name="ob", bufs=3) as opool,
            tc.tile_pool(name="ps", bufs=2, space="PSUM") as ppool,
        ):
            # inputs on the two HWDGE queues; outputs mostly on gpsimd
            in_e = [nc.sync, nc.scalar]
            in_rr = [0]

            def load(dst, src):
                in_e[in_rr[0] % 2].dma_start(dst, src)
                in_rr[0] += 1

            wa = wpool.tile([128, 27, COUT], dt_in, tag="wa")
            wb = wpool.tile([128, 9, COUT], dt_in, tag="wb")
            w1 = wpool.tile([128, 5, COUT], dt_in, tag="w1")

            xat = [
                xpool.tile([128, AALLOC], dt_in, name=f"xa{s}", tag=f"xa{s}")
                for s in range(NSLAB)
            ]
            xbt = [
                xpool.tile([128, BALLOC], dt_in, name=f"xb{s}", tag=f"xb{s}")
                for s in range(NSLAB)
            ]

            # thirds, aligned so tile-pair xi needs chunks {xi<=1: T0/T1,
            # xi>=2: T1/T2} of slabs w..w+2
            AT = [(0, 6 * PA), (6 * PA, 12 * PA), (12 * PA, 18 * PA)]
            BT = [(0, 6 * PB), (6 * PB, 12 * PB), (12 * PB, 18 * PB)]

            def load_a(s, t):
                c0, c1 = AT[t]
                load(xat[s][:, c0:c1], xa_h[:, s, c0:c1])

            def load_b(s, t):
                c0, c1 = BT[t]
                load(xbt[s][:, c0:c1], xb_h[:, s, c0:c1])

            load(wa[:], wa_h[:])
            for s in (0, 1, 2):
                load_a(s, 0)
            for s in (0, 1, 2):
                load_a(s, 1)
            load(wb[:], wb_h[:])
            load(w1[:], w1_h[:])
            for s in (0, 1, 2):
                load_b(s, 0)
                load_b(s, 1)
            for s in (0, 1, 2):
                load_a(s, 2)
                load_b(s, 2)
            for s in (3, 4, 5):
                for t in (0, 1, 2):
                    load_a(s, t)
                    load_b(s, t)

            def rhs_a(xt, prange, q0):
                v = xt[prange, q0 : q0 + 2 * PA]
                v = v.rearrange("p (x c) -> p x c", x=2, c=PA)
                return v[:, :, 0:256]

            def rhs_b(xt, prange, q0):
                v = xt[prange, q0 : q0 + 2 * PB]
                v = v.rearrange("p (x c) -> p x c", x=2, c=PB)
                return v[:, :, 0:256]

            PFULL = slice(0, 128)
            PLO = slice(0, 64)
            PHI = slice(64, 128)

            # ---- PE warmup on a memset scratch tile (no DMA dependency)
            scr = xpool.tile([128, 576], dt_in, name="scr", tag="scr")
            nc.gpsimd.memset(scr[:], 0.0)
            warm_ps = ppool.tile([128, 512], f32, tag="pqa")
            for _ in range(9):
                nc.tensor.matmul(
                    warm_ps[0:64, :], scr[:, 0:64], scr[:, 64:576],
                    start=True, stop=True, tile_position=(0, 0),
                )

            # ---- main loop: 8 groups of 2 tile-pairs ----
            groups = []
            for w in range(NW):
                groups.append([(w, 0), (w, 1)])
                groups.append([(w, 2), (w, 3)])

            n_tp = 0
            for g, tps in enumerate(groups):
                banks = []
                for t in range(len(tps)):
                    pq = ppool.tile([128, 512], f32, tag=f"pq{'ab'[t]}")
                    pr = ppool.tile([128, 512], f32, tag=f"pr{'ab'[t]}")
                    banks.append((pq, pr))

                # A-pairs: 27 K=128 matmuls per tile, offsets (dw,dx,dy,dz 0|1)
                for j in range(27):
                    dw, dx, dy = j // 9, (j // 3) % 3, j % 3
                    st = j == 0
                    for t, (w, xi) in enumerate(tps):
                        xt = xat[w + dw]
                        qa = (4 * xi + dx) * PA + dy * 16
                        pq = banks[t][0]
                        nc.tensor.matmul(
                            pq[0:64, :], wa[:, j, :], rhs_a(xt, PFULL, qa),
                            start=st, stop=False, tile_position=(0, 0),
                        )
                        nc.tensor.matmul(
                            pq[64:128, :], wa[:, j, :],
                            rhs_a(xt, PFULL, qa + 2 * PA),
                            start=st, stop=False, tile_position=(0, 64),
                        )

                # B-pairs: 9 K=128 matmuls per tile, (dw,dx, dy 0|1, dz=2)
                for m in range(9):
                    dw, dx = m // 3, m % 3
                    for t, (w, xi) in enumerate(tps):
                        xt = xbt[w + dw]
                        qa = (4 * xi + dx) * PB + 16
                        pq = banks[t][0]
                        nc.tensor.matmul(
                            pq[0:64, :], wb[:, m, :], rhs_b(xt, PFULL, qa),
                            start=False, stop=False, tile_position=(0, 0),
                        )
                        nc.tensor.matmul(
                            pq[64:128, :], wb[:, m, :],
                            rhs_b(xt, PFULL, qa + 2 * PB),
                            start=False, stop=False, tile_position=(0, 64),
                        )

                # B-singles: (dw,dx, dy=2, dz=2), K=64 quad-packed.  The
                # last group runs tile-pair-major so the first tile-pair's
                # banks stop ~1us early and its epilogue overlaps the rest.
                def emit_singles(t, i):
                    w, xi = tps[t]
                    pq, pr = banks[t]
                    me = 2 * i
                    mo = me + 1
                    dw, dx = me // 3, me % 3
                    xt = xbt[w + dw]
                    qe = (4 * xi + dx) * PB + 48
                    stop0 = i == 4
                    nc.tensor.matmul(
                        pq[0:64, :], w1[0:64, i, :], rhs_b(xt, PLO, qe),
                        start=False, stop=stop0, tile_position=(0, 0),
                    )
                    nc.tensor.matmul(
                        pq[64:128, :], w1[0:64, i, :],
                        rhs_b(xt, PLO, qe + 2 * PB),
                        start=False, stop=stop0, tile_position=(0, 64),
                    )
                    if mo < 9:
                        dw, dx = mo // 3, mo % 3
                        xt = xbt[w + dw]
                        qo = (4 * xi + dx) * PB + 32
                        st1 = i == 0
                        stop1 = i == 3
                        nc.tensor.matmul(
                            pr[0:64, :], w1[64:128, i, :],
                            rhs_b(xt, PHI, qo),
                            start=st1, stop=stop1,
                            tile_position=(64, 0),
                        )
                        nc.tensor.matmul(
                            pr[64:128, :], w1[64:128, i, :],
                            rhs_b(xt, PHI, qo + 2 * PB),
                            start=st1, stop=stop1,
                            tile_position=(64, 64),
                        )

                if g == len(groups) - 1:
                    for t in range(len(tps)):
                        for i in range(5):
                            emit_singles(t, i)
                else:
                    for i in range(5):
                        for t in range(len(tps)):
                            emit_singles(t, i)

                # epilogue: osb = pr (ACT copy) + pq (DVE add), then DMA out
                for t, (w, xi) in enumerate(tps):
                    pq, pr = banks[t]
                    n_tp += 1
                    x0 = 4 * xi
                    osb = opool.tile([128, 512], dt_in, tag="osb")
                    nc.scalar.copy(osb[:, :], pr[:, :])
                    nc.vector.tensor_add(
                        out=osb[:, :], in0=pq[:, :], in1=osb[:, :]
                    )
                    lo = osb[0:64, :].rearrange(
                        "p (x y z) -> p x y z", x=2, y=16, z=16
                    )
                    hi = osb[64:128, :].rearrange(
                        "p (x y z) -> p x y z", x=2, y=16, z=16
                    )
                    if n_tp == 16:
                        nc.sync.dma_start(out_h[:, w, x0 : x0 + 2, :, :], lo)
                        nc.scalar.dma_start(
                            out_h[:, w, x0 + 2 : x0 + 4, :, :], hi
                        )
                    else:
                        nc.gpsimd.dma_start(out_h[:, w, x0 : x0 + 2, :, :], lo)
                        nc.gpsimd.dma_start(
                            out_h[:, w, x0 + 2 : x0 + 4, :, :], hi
                        )

    _dedup_ldweights(nc)
    _split_multiwaits(nc)
    return nc


def _dedup_ldweights(nc):
    """bass lowers every matmul to a standalone InstLdweights + a
    non-self-loading InstMatmult.  Consecutive matmuls in a group share the
    same stationary weights per PE quadrant, so half those loads reload
    identical data.  Delete an InstLdweights when the same (AP, quadrant)
    load is still resident and nothing overlapping was loaded in between.
    Loads carrying sync waits/updates are kept (sem graph untouched)."""
    removed = 0
    for fn in nc.m.functions:
        for blk in fn.blocks:
            resident = {}  # (r0, c0, r1, c1) -> signature
            out = []
            for inst in blk.instructions:
                if type(inst).__name__ != "InstLdweights":
                    out.append(inst)
                    continue
                pos = inst.tile_position or (0, 0)
                ts = inst.tile_size or (128, 128)
                reg = (pos[0], pos[1], pos[0] + ts[0], pos[1] + ts[1])
                sig = (repr(inst.ins[0]), pos, ts, inst.perf_mode)
                si = inst.sync_info
                clean = si is None or (not si.on_wait and not si.on_update)
                if clean and resident.get(reg) == sig:
                    removed += 1
                    continue
                # invalidate anything overlapping this load's region
                for r in list(resident):
                    if (
                        r[0] < reg[2] and reg[0] < r[2]
                        and r[1] < reg[3] and reg[1] < r[3]
                    ):
                        del resident[r]
                resident[reg] = sig
                out.append(inst)
            blk.instructions = out
    return removed


def _make_tile_context(nc):
    from concourse.tile import TileContext

    class TC(TileContext):
        # stock teardown is drain -> barrier -> sem-clear -> barrier; the
        # final barrier only orders engine-stream ends and costs ~2us.
        def _drain_and_barrier(self, tick_clock, wait_clock):
            from concourse.vector_clock import ScopedClock

            nc = self.nc
            drain_inst = nc.sync.drain()
            wait_clock.add_sem_waits(
                drain_inst.ins, ScopedClock({None: tick_clock.global_clock})
            )
            nc.all_engine_barrier()
            assert self.sems is not None
            popped = nc._tile_sem_poison_stack.pop()
            assert popped is self._sem_poison
            nc.clear_and_free_semaphores(list(self.sems.allocated().values()))

    return TC(nc)


def _split_multiwaits(nc, max_waits=1):
    """The walrus build here rejects any instruction carrying more than one
    sync-wait ("Too many sync wait commands").  Tile attaches one wait per
    outstanding producer.  Move excess waits onto same-engine NoOps inserted
    immediately before the instruction - semantically identical."""
    import concourse.mybir as mybir

    n_split = 0
    for fn in nc.m.functions:
        for blk in fn.blocks:
            out = []
            for inst in list(blk.instructions):
                si = inst.sync_info
                if si is not None and si.on_wait and len(si.on_wait) > max_waits:
                    waits = list(si.on_wait)
                    extra = waits[:-max_waits]
                    for k in range(0, len(extra), max_waits):
                        nop = mybir.InstNoOp(
                            name=f"{inst.name}.w{k}", ins=[], outs=[]
                        )
                        nop.engine = inst.engine
                        nop.sync_info = mybir.SyncInfo(
                            on_wait=extra[k : k + max_waits], on_update=[]
                        )
                        nc.register_instruction(nop)
                        out.append(nop)
                        n_split += 1
                    si.on_wait = waits[-max_waits:]
                out.append(inst)
            blk.instructions = out
    return n_split


# compute dtype: "float16" (fastest, rel err ~3e-4)
DTYPE = "float16"


def _get_nc():
    if "nc" not in _CACHE:
        import concourse.mybir as mybir

        _CACHE["nc"] = _build_nc(getattr(mybir.dt, DTYPE))
    return _CACHE["nc"]


def _np_dtype():
    if DTYPE == "float16":
        return np.float16
    return np.float32


def _shard_inputs(inputs):
    nd = _np_dtype()
    x = np.asarray(inputs["inputs"], dtype=np.float32).astype(nd)
    wk = np.asarray(inputs["kernel"], dtype=np.float32).astype(nd)
    k4 = wk.reshape(3, 3, 3, 3, CIN, COUT)  # [dw,dx,dy,dz,ci,co]

    # layout A weights: rows 0-63 = dz=1 tap, rows 64-127 = dz=0 tap
    wa = np.empty((128, 27, COUT), dtype=nd)
    wa[0:64] = k4[:, :, :, 1].reshape(27, CIN, COUT).transpose(1, 0, 2)
    wa[64:128] = k4[:, :, :, 0].reshape(27, CIN, COUT).transpose(1, 0, 2)
    # layout B pair weights: rows 0-63 = (dy=0,dz=2), rows 64-127 = (dy=1,dz=2)
    wb = np.empty((128, 9, COUT), dtype=nd)
    wb[0:64] = k4[:, :, 0, 2].reshape(9, CIN, COUT).transpose(1, 0, 2)
    wb[64:128] = k4[:, :, 1, 2].reshape(9, CIN, COUT).transpose(1, 0, 2)
    # B singles (dy=2,dz=2): even tap index on rows 0-63, odd on rows 64-127
    w22 = k4[:, :, 2, 2].reshape(9, CIN, COUT)
    w1 = np.zeros((128, 5, COUT), dtype=nd)
    w1[0:64] = w22[0::2].transpose(1, 0, 2)
    w1[64:128, 0:4] = w22[1::2].transpose(1, 0, 2)

    in_maps = []
    for c in range(8):
        b, wc = c // 4, c % 4
        w0 = 4 * wc
        slab = x[b, :, w0 : w0 + 6]  # [64, 6, 18, 18, 18] (w, x, y, z)
        # layout A: z compacted to 16; lo rows z-base 1, hi rows z-base 0
        xa = np.empty((128, NSLAB, ACOLS), dtype=nd)
        xa[0:64] = slab[..., 1:17].reshape(CIN, NSLAB, ACOLS)
        xa[64:128] = slab[..., 0:16].reshape(CIN, NSLAB, ACOLS)
        # layout B: z-base 2 compacted, 19-row y panels; lo rows shifted +1 y
        cb = slab[..., 2:18]  # [64, 6, 18, 18, 16]
        lo = np.zeros((CIN, NSLAB, S, 19, 16), dtype=nd)
        hi = np.zeros((CIN, NSLAB, S, 19, 16), dtype=nd)
        lo[:, :, :, 1:19] = cb
        hi[:, :, :, 0:18] = cb
        xb = np.empty((128, NSLAB, BCOLS), dtype=nd)
        xb[0:64] = lo.reshape(CIN, NSLAB, BCOLS)
        xb[64:128] = hi.reshape(CIN, NSLAB, BCOLS)
        in_maps.append({"xa": xa, "xb": xb, "wa": wa, "wb": wb, "w1": w1})
    return in_maps


def _gather_outputs(results):
    out = np.empty((B, COUT, NW * 4, SO, SO, SO), dtype=np.float32)
    for c in range(8):
        b, wc = c // 4, c % 4
        w0 = 4 * wc
        out[b, :, w0 : w0 + 4] = results[c]["out"]
    return out


def kernel(**inputs):
    from concourse.bass_utils import run_bass_kernel_spmd

    res = run_bass_kernel_spmd(_get_nc(), _shard_inputs(inputs), list(range(8)))
    return _gather_outputs(res.results)


# revision 11
# speedup vs baseline: 1.1467x; 1.0063x over previous
"""Trainium2 Bass kernel for 4-D valid convolution.

Problem: inputs [2, 64, 18, 18, 18, 18] fp32, kernel [81, 64, 64] fp32
(81 = 3^4 offsets row-major over (dw, dx, dy, dz)), output
[2, 64, 16, 16, 16, 16] fp32.

Sharding (8 cores): batch (2) x output-W chunks (4 chunks of 4).  Each core
receives an input slab x[b, :, w0:w0+6] plus the full kernel, and produces
out[b, :, w0:w0+4] as [64, 4, 16, 16, 16].

All matmul reads are CONTIGUOUS via two host-compacted layouts per slab
(measured on HW: a strided conv window costs +27 ns per 512-col slot; a
contiguous one runs at the 215.8 ns floor):

  Layout A [128, 18x * (18y*16z)]: rows 64-127 hold x(..., z+0) z-compacted
  to 16, rows 0-63 hold x(..., z+1).  One K=128 matmul at column base
  ((x0+dx)*288 + dy*16) covers the offset pair (dz=0, dz=1) for tap
  (dw,dx,dy) with a fully contiguous [p, 2x, 256] read.  27 taps.

  Layout B [128, 18x * (19y'*16z)]: z-base 2 compacted (cb = x(...,z+2)),
  rows 64-127 = cb(y'), rows 0-63 = cb(y'-1) (one-row y shift, 19-row
  panels so the shift never clips).  One K=128 matmul at y-base 1 covers
  the pair (dy=0,dz=2)+(dy=1,dz=2) per (dw,dx): 9 taps-pairs.  The 9
  leftover (dy=2,dz=2) taps run as K=64 quad-packed singles (even tap on
  rows 0-63 at y-base 3, odd on rows 64-127 at y-base 2).

Per output tile-pair (1024 positions = 4 x-values x 16y x 16z, split
qa/qb across PE column groups 0/64): 27 + 9 K=128 slots + ~5 quad slots,
one PSUM bank for pairs+even-singles (pq), one for odd singles (pr).
Groups of 2 tile-pairs share LDWEIGHTS (a post-pass deletes reloads of
identical resident weights; measured: <=1 LDW per 512-col slot is free).

Scheduling: slab tiles with column-chunked DMA (dep tracking is
range-based), loads ordered by first use; ~12 warmup matmuls on a memset
scratch tile un-throttle the PE clock (1.2 -> 2.4 GHz) during the DMA
prologue; output DMAs ride the otherwise-idle gpsimd queue except the
last tile-pair's, which use the low-latency HWDGE queues.
"""

import numpy as np

B, CIN, COUT = 2, 64, 64
S = 18          # input spatial per dim
SO = 16         # output spatial per dim
NW = 4          # output w per core
NSLAB = 6       # input w slabs per core
PA = 288        # layout A x-panel: 18y * 16z
PB = 304        # layout B x-panel: 19y' * 16z
ACOLS = 18 * PA          # 5184
BCOLS = 18 * PB          # 5472
AALLOC = ACOLS + 40      # view-extent padding (max extent 5216)
BALLOC = BCOLS + 52      # max extent 5520

_CACHE = {}


def _build_nc(dt_in):
    import concourse.bass as bass
    import concourse.mybir as mybir

    f32 = mybir.dt.float32

    nc = bass.Bass()
    xa_h = nc.dram_tensor("xa", [128, NSLAB, ACOLS], dt_in, kind="ExternalInput")
    xb_h = nc.dram_tensor("xb", [128, NSLAB, BCOLS], dt_in, kind="ExternalInput")
    wa_h = nc.dram_tensor("wa", [128, 27, COUT], dt_in, kind="ExternalInput")
    wb_h = nc.dram_tensor("wb", [128, 9, COUT], dt_in, kind="ExternalInput")
    w1_h = nc.dram_tensor("w1", [128, 5, COUT], dt_in, kind="ExternalInput")
    out_h = nc.dram_tensor(
        "out", [COUT, NW, SO, SO, SO], dt_in, kind="ExternalOutput"
    )

    tc = _make_tile_context(nc)
    with tc:
        with (
            tc.tile_pool(name="xp", bufs=1) as xpool,
            tc.tile_pool(name="wp", bufs=1) as wpool,
            tc.tile_pool(name="ob", bufs=3) as opool,
            tc.tile_pool(name="ps", bufs=2, space="PSUM") as ppool,
        ):
            # inputs on the two HWDGE queues; outputs mostly on gpsimd
            in_e = [nc.sync, nc.scalar]
            in_rr = [0]

            def load(dst, src):
                in_e[in_rr[0] % 2].dma_start(dst, src)
                in_rr[0] += 1

            wa = wpool.tile([128, 27, COUT], dt_in, tag="wa")
            wb = wpool.tile([128, 9, COUT], dt_in, tag="wb")
            w1 = wpool.tile([128, 5, COUT], dt_in, tag="w1")

            xat = [
                xpool.tile([128, AALLOC], dt_in, name=f"xa{s}", tag=f"xa{s}")
                for s in range(NSLAB)
            ]
            xbt = [
                xpool.tile([128, BALLOC], dt_in, name=f"xb{s}", tag=f"xb{s}")
                for s in range(NSLAB)
            ]

            # thirds, aligned so tile-pair xi needs chunks {xi<=1: T0/T1,
            # xi>=2: T1/T2} of slabs w..w+2
            AT = [(0, 6 * PA), (6 * PA, 12 * PA), (12 * PA, 18 * PA)]
            BT = [(0, 6 * PB), (6 * PB, 12 * PB), (12 * PB, 18 * PB)]

            def load_a(s, t):
                c0, c1 = AT[t]
                load(xat[s][:, c0:c1], xa_h[:, s, c0:c1])

            def load_b(s, t):
                c0, c1 = BT[t]
                load(xbt[s][:, c0:c1], xb_h[:, s, c0:c1])

            load(wa[:], wa_h[:])
            for s in (0, 1, 2):
                load_a(s, 0)
            for s in (0, 1, 2):
                load_a(s, 1)
            load(wb[:], wb_h[:])
            load(w1[:], w1_h[:])
            for s in (0, 1, 2):
                load_b(s, 0)
                load_b(s, 1)
            for s in (0, 1, 2):
                load_a(s, 2)
                load_b(s, 2)
            for s in (3, 4, 5):
                for t in (0, 1, 2):
                    load_a(s, t)
                    load_b(s, t)

            def rhs_a(xt, prange, q0):
                v = xt[prange, q0 : q0 + 2 * PA]
                v = v.rearrange("p (x c) -> p x c", x=2, c=PA)
                return v[:, :, 0:256]

            def rhs_b(xt, prange, q0):
                v = xt[prange, q0 : q0 + 2 * PB]
                v = v.rearrange("p (x c) -> p x c", x=2, c=PB)
                return v[:, :, 0:256]

            PFULL = slice(0, 128)
            PLO = slice(0, 64)
            PHI = slice(64, 128)

            # ---- PE warmup on a memset scratch tile (no DMA dependency)
            scr = xpool.tile([128, 576], dt_in, name="scr", tag="scr")
            nc.gpsimd.memset(scr[:], 0.0)
            warm_ps = ppool.tile([128, 512], f32, tag="pqa")
            for _ in range(9):
                nc.tensor.matmul(
                    warm_ps[0:64, :], scr[:, 0:64], scr[:, 64:576],
                    start=True, stop=True, tile_position=(0, 0),
                )

            # ---- main loop: 8 groups of 2 tile-pairs ----
            groups = []
            for w in range(NW):
                groups.append([(w, 0), (w, 1)])
                groups.append([(w, 2), (w, 3)])

            n_tp = 0
            for g, tps in enumerate(groups):
                banks = []
                for t in range(len(tps)):
                    pq = ppool.tile([128, 512], f32, tag=f"pq{'ab'[t]}")
                    pr = ppool.tile([128, 512], f32, tag=f"pr{'ab'[t]}")
                    banks.append((pq, pr))

                # A-pairs: 27 K=128 matmuls per tile, offsets (dw,dx,dy,dz 0|1).
                # Group 0 runs dx-major: its dx=0 taps need only the first
                # column-third of slabs 0-2, which lands ~6us before the rest.
                if g == 0:
                    jorder = sorted(
                        range(27), key=lambda j: ((j // 3) % 3, j // 9, j % 3)
                    )
                else:
                    jorder = list(range(27))
                for jn, j in enumerate(jorder):
                    dw, dx, dy = j // 9, (j // 3) % 3, j % 3
                    st = jn == 0
                    for t, (w, xi) in enumerate(tps):
                        xt = xat[w + dw]
                        qa = (4 * xi + dx) * PA + dy * 16
                        pq = banks[t][0]
                        nc.tensor.matmul(
                            pq[0:64, :], wa[:, j, :], rhs_a(xt, PFULL, qa),
                            start=st, stop=False, tile_position=(0, 0),
                        )
                        nc.tensor.matmul(
                            pq[64:128, :], wa[:, j, :],
                            rhs_a(xt, PFULL, qa + 2 * PA),
                            start=st, stop=False, tile_position=(0, 64),
                        )

                # B-pairs: 9 K=128 matmuls per tile, (dw,dx, dy 0|1, dz=2)
                for m in range(9):
                    dw, dx = m // 3, m % 3
                    for t, (w, xi) in enumerate(tps):
                        xt = xbt[w + dw]
                        qa = (4 * xi + dx) * PB + 16
                        pq = banks[t][0]
                        nc.tensor.matmul(
                            pq[0:64, :], wb[:, m, :], rhs_b(xt, PFULL, qa),
                            start=False, stop=False, tile_position=(0, 0),
                        )
                        nc.tensor.matmul(
                            pq[64:128, :], wb[:, m, :],
                            rhs_b(xt, PFULL, qa + 2 * PB),
                            start=False, stop=False, tile_position=(0, 64),
                        )

                # B-singles: (dw,dx, dy=2, dz=2), K=64 quad-packed.  The
                # last group runs tile-pair-major so the first tile-pair's
                # banks stop ~1us early and its epilogue overlaps the rest.
                def emit_singles(t, i):
                    w, xi = tps[t]
                    pq, pr = banks[t]
                    me = 2 * i
                    mo = me + 1
                    dw, dx = me // 3, me % 3
                    xt = xbt[w + dw]
                    qe = (4 * xi + dx) * PB + 48
                    stop0 = i == 4
                    nc.tensor.matmul(
                        pq[0:64, :], w1[0:64, i, :], rhs_b(xt, PLO, qe),
                        start=False, stop=stop0, tile_position=(0, 0),
                    )
                    nc.tensor.matmul(
                        pq[64:128, :], w1[0:64, i, :],
                        rhs_b(xt, PLO, qe + 2 * PB),
                        start=False, stop=stop0, tile_position=(0, 64),
                    )
                    if mo < 9:
                        dw, dx = mo // 3, mo % 3
                        xt = xbt[w + dw]
                        qo = (4 * xi + dx) * PB + 32
                        st1 = i == 0
                        stop1 = i == 3
                        nc.tensor.matmul(
                            pr[0:64, :], w1[64:128, i, :],
                            rhs_b(xt, PHI, qo),
                            start=st1, stop=stop1,
                            tile_position=(64, 0),
                        )
                        nc.tensor.matmul(
                            pr[64:128, :], w1[64:128, i, :],
                            rhs_b(xt, PHI, qo + 2 * PB),
                            start=st1, stop=stop1,
                            tile_position=(64, 64),
                        )

                if g == len(groups) - 1:
                    for t in range(len(tps)):
                        for i in range(5):
                            emit_singles(t, i)
                else:
                    for i in range(5):
                        for t in range(len(tps)):
                            emit_singles(t, i)

                # epilogue: osb = pr (ACT copy) + pq (DVE add), then DMA out
                for t, (w, xi) in enumerate(tps):
                    pq, pr = banks[t]
                    n_tp += 1
                    x0 = 4 * xi
                    osb = opool.tile([128, 512], dt_in, tag="osb")
                    nc.scalar.copy(osb[:, :], pr[:, :])
                    nc.vector.tensor_add(
                        out=osb[:, :], in0=pq[:, :], in1=osb[:, :]
                    )
                    lo = osb[0:64, :].rearrange(
                        "p (x y z) -> p x y z", x=2, y=16, z=16
                    )
                    hi = osb[64:128, :].rearrange(
                        "p (x y z) -> p x y z", x=2, y=16, z=16
                    )
                    if n_tp == 16:
                        nc.sync.dma_start(out_h[:, w, x0 : x0 + 2, :, :], lo)
                        nc.scalar.dma_start(
                            out_h[:, w, x0 + 2 : x0 + 4, :, :], hi
                        )
                    else:
                        nc.gpsimd.dma_start(out_h[:, w, x0 : x0 + 2, :, :], lo)
                        nc.gpsimd.dma_start(
                            out_h[:, w, x0 + 2 : x0 + 4, :, :], hi
                        )

    _dedup_ldweights(nc)
    _split_multiwaits(nc)
    return nc


def _dedup_ldweights(nc):
    """bass lowers every matmul to a standalone InstLdweights + a
    non-self-loading InstMatmult.  Consecutive matmuls in a group share the
    same stationary weights per PE quadrant, so half those loads reload
    identical data.  Delete an InstLdweights when the same (AP, quadrant)
    load is still resident and nothing overlapping was loaded in between.
    Loads carrying sync waits/updates are kept (sem graph untouched)."""
    removed = 0
    for fn in nc.m.functions:
        for blk in fn.blocks:
            resident = {}  # (r0, c0, r1, c1) -> signature
            out = []
            for inst in blk.instructions:
                if type(inst).__name__ != "InstLdweights":
                    out.append(inst)
                    continue
                pos = inst.tile_position or (0, 0)
                ts = inst.tile_size or (128, 128)
                reg = (pos[0], pos[1], pos[0] + ts[0], pos[1] + ts[1])
                sig = (repr(inst.ins[0]), pos, ts, inst.perf_mode)
                si = inst.sync_info
                clean = si is None or (not si.on_wait and not si.on_update)
                if clean and resident.get(reg) == sig:
                    removed += 1
                    continue
                # invalidate anything overlapping this load's region
                for r in list(resident):
                    if (
                        r[0] < reg[2] and reg[0] < r[2]
                        and r[1] < reg[3] and reg[1] < r[3]
                    ):
                        del resident[r]
                resident[reg] = sig
                out.append(inst)
            blk.instructions = out
    return removed


def _make_tile_context(nc):
    from concourse.tile import TileContext

    class TC(TileContext):
        # stock teardown is drain -> barrier -> sem-clear -> barrier; the
        # final barrier only orders engine-stream ends and costs ~2us.
        def _drain_and_barrier(self, tick_clock, wait_clock):
            from concourse.vector_clock import ScopedClock

            nc = self.nc
            drain_inst = nc.sync.drain()
            wait_clock.add_sem_waits(
                drain_inst.ins, ScopedClock({None: tick_clock.global_clock})
            )
            nc.all_engine_barrier()
            assert self.sems is not None
            popped = nc._tile_sem_poison_stack.pop()
            assert popped is self._sem_poison
            nc.clear_and_free_semaphores(list(self.sems.allocated().values()))

    return TC(nc)


def _split_multiwaits(nc, max_waits=1):
    """The walrus build here rejects any instruction carrying more than one
    sync-wait ("Too many sync wait commands").  Tile attaches one wait per
    outstanding producer.  Move excess waits onto same-engine NoOps inserted
    immediately before the instruction - semantically identical."""
    import concourse.mybir as mybir

    n_split = 0
    for fn in nc.m.functions:
        for blk in fn.blocks:
            out = []
            for inst in list(blk.instructions):
                si = inst.sync_info
                if si is not None and si.on_wait and len(si.on_wait) > max_waits:
                    waits = list(si.on_wait)
                    extra = waits[:-max_waits]
                    for k in range(0, len(extra), max_waits):
                        nop = mybir.InstNoOp(
                            name=f"{inst.name}.w{k}", ins=[], outs=[]
                        )
                        nop.engine = inst.engine
                        nop.sync_info = mybir.SyncInfo(
                            on_wait=extra[k : k + max_waits], on_update=[]
                        )
                        nc.register_instruction(nop)
                        out.append(nop)
                        n_split += 1
                    si.on_wait = waits[-max_waits:]
                out.append(inst)
            blk.instructions = out
    return n_split


# compute dtype: "float16" (fastest, rel err ~3e-4)
DTYPE = "float16"


def _get_nc():
    if "nc" not in _CACHE:
        import concourse.mybir as mybir

        _CACHE["nc"] = _build_nc(getattr(mybir.dt, DTYPE))
    return _CACHE["nc"]


def _np_dtype():
    if DTYPE == "float16":
        return np.float16
    return np.float32


def _shard_inputs(inputs):
    nd = _np_dtype()
    x = np.asarray(inputs["inputs"], dtype=np.float32).astype(nd)
    wk = np.asarray(inputs["kernel"], dtype=np.float32).astype(nd)
    k4 = wk.reshape(3, 3, 3, 3, CIN, COUT)  # [dw,dx,dy,dz,ci,co]

    # layout A weights: rows 0-63 = dz=1 tap, rows 64-127 = dz=0 tap
    wa = np.empty((128, 27, COUT), dtype=nd)
    wa[0:64] = k4[:, :, :, 1].reshape(27, CIN, COUT).transpose(1, 0, 2)
    wa[64:128] = k4[:, :, :, 0].reshape(27, CIN, COUT).transpose(1, 0, 2)
    # layout B pair weights: rows 0-63 = (dy=0,dz=2), rows 64-127 = (dy=1,dz=2)
    wb = np.empty((128, 9, COUT), dtype=nd)
    wb[0:64] = k4[:, :, 0, 2].reshape(9, CIN, COUT).transpose(1, 0, 2)
    wb[64:128] = k4[:, :, 1, 2].reshape(9, CIN, COUT).transpose(1, 0, 2)
    # B singles (dy=2,dz=2): even tap index on rows 0-63, odd on rows 64-127
    w22 = k4[:, :, 2, 2].reshape(9, CIN, COUT)
    w1 = np.zeros((128, 5, COUT), dtype=nd)
    w1[0:64] = w22[0::2].transpose(1, 0, 2)
    w1[64:128, 0:4] = w22[1::2].transpose(1, 0, 2)

    in_maps = []
    for c in range(8):
        b, wc = c // 4, c % 4
        w0 = 4 * wc
        slab = x[b, :, w0 : w0 + 6]  # [64, 6, 18, 18, 18] (w, x, y, z)
        # layout A: z compacted to 16; lo rows z-base 1, hi rows z-base 0
        xa = np.empty((128, NSLAB, ACOLS), dtype=nd)
        xa[0:64] = slab[..., 1:17].reshape(CIN, NSLAB, ACOLS)
        xa[64:128] = slab[..., 0:16].reshape(CIN, NSLAB, ACOLS)
        # layout B: z-base 2 compacted, 19-row y panels; lo rows shifted +1 y
        cb = slab[..., 2:18]  # [64, 6, 18, 18, 16]
        lo = np.zeros((CIN, NSLAB, S, 19, 16), dtype=nd)
        hi = np.zeros((CIN, NSLAB, S, 19, 16), dtype=nd)
        lo[:, :, :, 1:19] = cb
        hi[:, :, :, 0:18] = cb
        xb = np.empty((128, NSLAB, BCOLS), dtype=nd)
        xb[0:64] = lo.reshape(CIN, NSLAB, BCOLS)
        xb[64:128] = hi.reshape(CIN, NSLAB, BCOLS)
        in_maps.append({"xa": xa, "xb": xb, "wa": wa, "wb": wb, "w1": w1})
    return in_maps


def _gather_outputs(results):
    out = np.empty((B, COUT, NW * 4, SO, SO, SO), dtype=np.float32)
    for c in range(8):
        b, wc = c // 4, c % 4
        w0 = 4 * wc
        out[b, :, w0 : w0 + 4] = results[c]["out"]
    return out


def kernel(**inputs):
    from concourse.bass_utils import run_bass_kernel_spmd

    res = run_bass_kernel_spmd(_get_nc(), _shard_inputs(inputs), list(range(8)))
    return _gather_outputs(res.results)
